# revision 9
# baseline (speedup 1.0000x reference)
"""Trainium2 Bass kernel for nn_DecoderLayer (self-attn + cross-attn + FFN).

Sharding: data-parallel over batch. 16 batches / 8 cores = 2 per core, all
weights replicated, no collectives.

Per-core dataflow (everything in "X layout" [D(part), T(free)], fp16 matmul
operands, fp32 PSUM/layernorm math):
  - inputs transposed once via fp16 DRAM bounce + xbar DMA transpose
  - Q^T/K^T projections keep D on partitions; V projected to natural layout
  - attention logits computed in BOTH orientations per head (cheaper than
    transposing the 512x512 attention matrices):
      E^T[k, q] = exp(logits^T) feeds attn@V;  aw[q, k] is the output pass,
      normalized for free via exp(L - ln S) with a per-partition bias
  - S = sum_k E^T via one-hot matmul reduction; 1/S broadcast via pair-select
    matmul; layernorm stats via ones-column matmul reduction
  - heads processed in 2 waves of 4 to bound SBUF usage
"""

import numpy as np
from contextlib import ExitStack

import concourse.bass as bass
import concourse.tile as tile
from concourse import bacc, mybir
from concourse.bass import ts
from concourse.bass_utils import run_bass_kernel_spmd
from concourse.masks import make_identity

P = 128
B, T, D, H, DH, DFF = 16, 512, 512, 8, 64, 2048
NCORES = 8
BPC = B // NCORES
DC = D // P       # 4
FC = DFF // P     # 16
WAVE = 4          # heads per wave
EPS = 1e-6

F16 = mybir.dt.float16
F32 = mybir.dt.float32
AF = mybir.ActivationFunctionType
OP = mybir.AluOpType

WNAMES = [f"{p}_{w}" for p in ("m1", "m2") for w in ("wq", "wk", "wv", "do")]


class Ctx:
    pass


def _load_col(c, dram_vec, n_chunks, tag):
    t = c.wp.tile([P, n_chunks], F32, tag=tag)
    c.nc.sync.dma_start(t[:], dram_vec.rearrange("(c p) -> p c", p=P))
    return t


def _load_row16(c, dram_vec, tag, scale=None):
    nc = c.nc
    tf = c.smp.tile([1, T], F32, tag="rowf")
    nc.sync.dma_start(tf[:], dram_vec[None, :])
    t = c.wp.tile([1, T], F16, tag=tag)
    if scale is None:
        nc.vector.tensor_copy(t[:], tf[:])
    else:
        nc.vector.tensor_scalar_mul(t[:], tf[:], scale)
    return t


def build(mask_zero: bool, pad_zero: bool):
    nc = bacc.Bacc("TRN2", target_bir_lowering=False, debug=False)
    io = {}
    io["x"] = nc.dram_tensor("x", [BPC, T, D], F32, kind="ExternalInput").ap()
    io["enc_fc"] = nc.dram_tensor("enc_fc", [BPC, T, D], F32, kind="ExternalInput").ap()
    io["enc_ff"] = nc.dram_tensor("enc_ff", [BPC, T, D], F32, kind="ExternalInput").ap()
    io["lam"] = nc.dram_tensor("lam", [T, T], F32, kind="ExternalInput").ap()
    io["pad"] = nc.dram_tensor("pad", [BPC, T], F32, kind="ExternalInput").ap()
    for w in WNAMES:
        io[w + "_w"] = nc.dram_tensor(w + "_w", [D, D], F32, kind="ExternalInput").ap()
        io[w + "_b"] = nc.dram_tensor(w + "_b", [D], F32, kind="ExternalInput").ap()
    io["ffn_w1"] = nc.dram_tensor("ffn_w1", [D, DFF], F32, kind="ExternalInput").ap()
    io["ffn_b1"] = nc.dram_tensor("ffn_b1", [DFF], F32, kind="ExternalInput").ap()
    io["ffn_w2"] = nc.dram_tensor("ffn_w2", [DFF, D], F32, kind="ExternalInput").ap()
    io["ffn_b2"] = nc.dram_tensor("ffn_b2", [D], F32, kind="ExternalInput").ap()
    for ln in ("ln1", "ln2", "ln3"):
        io[ln + "_g"] = nc.dram_tensor(ln + "_g", [D], F32, kind="ExternalInput").ap()
        io[ln + "_b"] = nc.dram_tensor(ln + "_b", [D], F32, kind="ExternalInput").ap()
    out3 = nc.dram_tensor("out3", [BPC, T, D], F32, kind="ExternalOutput").ap()
    aw1 = nc.dram_tensor("aw1", [BPC, H, T, T], F32, kind="ExternalOutput").ap()
    aw2 = nc.dram_tensor("aw2", [BPC, H, T, T], F32, kind="ExternalOutput").ap()

    with tile.TileContext(nc) as tc, ExitStack() as ctx:
        c = Ctx()
        c.nc, c.tc = nc, tc
        c.mask_zero, c.pad_zero = mask_zero, pad_zero

        # SBUF pools
        c.wp = ctx.enter_context(tc.tile_pool(name="wp", bufs=1))
        c.ws = ctx.enter_context(tc.tile_pool(name="ws", bufs=2))
        c.actT = ctx.enter_context(tc.tile_pool(name="actT", bufs=4))
        c.vp = ctx.enter_context(tc.tile_pool(name="vp", bufs=4))
        c.otp = ctx.enter_context(tc.tile_pool(name="otp", bufs=4))
        c.s16 = ctx.enter_context(tc.tile_pool(name="s16", bufs=8))
        c.e16 = ctx.enter_context(tc.tile_pool(name="e16", bufs=18))
        c.f32s = ctx.enter_context(tc.tile_pool(name="f32s", bufs=12))
        c.awp = ctx.enter_context(tc.tile_pool(name="awp", bufs=6))
        c.smp = ctx.enter_context(tc.tile_pool(name="smp", bufs=4))
        c.tcolp = ctx.enter_context(tc.tile_pool(name="tcolp", bufs=5))
        c.dram = ctx.enter_context(tc.tile_pool(name="dram", bufs=2, space="DRAM"))
        # PSUM pools: 4 + 3 + 1 = 8 banks
        c.pmm = ctx.enter_context(tc.tile_pool(name="pmm", bufs=4, space="PSUM"))
        c.psm = ctx.enter_context(tc.tile_pool(name="psm", bufs=3, space="PSUM"))
        c.pbc = ctx.enter_context(tc.tile_pool(name="pbc", bufs=1, space="PSUM"))

        _consts(c, io)
        _weights(c, io)

        for b in range(BPC):
            with nc.named_scope(f"b{b}_inT"):
                xT = _load_T(c, io["x"][b], "xT")
                fcT = _load_T(c, io["enc_fc"][b], "fcT")
                ffT = _load_T(c, io["enc_ff"][b], "ffT")
            if not pad_zero:
                pad_nr = _load_row16(c, io["pad"][b], "padnr", scale=-30000.0)
                pc_f = c.smp.tile([P, DC], F32, tag="padnc_l")
                nc.sync.dma_start(
                    pc_f[:], io["pad"][b].rearrange("(c p) -> p c", p=P))
                pad_ncol = c.wp.tile([P, DC], F32, tag="padnc")
                nc.vector.tensor_scalar_mul(pad_ncol[:], pc_f[:], -30000.0)
            else:
                pad_nr = pad_ncol = None
            with nc.named_scope(f"b{b}_v1"):
                v1 = _vproj(c, xT, c.w16["m1_wv"], c.bvrow["m1_wv"])
            with nc.named_scope(f"b{b}_attn1"):
                g1 = _attention(c, b, xT, xT, v1, "m1", aw1, True, None, None)
            with nc.named_scope(f"b{b}_ln1"):
                o1T = _layernorm(c, g1, xT, "ln1", F16)
            with nc.named_scope(f"b{b}_v2"):
                v2 = _vproj(c, ffT, c.w16["m2_wv"], c.bvrow["m2_wv"])
            with nc.named_scope(f"b{b}_attn2"):
                g2 = _attention(c, b, o1T, fcT, v2, "m2", aw2, False,
                                pad_nr, pad_ncol)
            with nc.named_scope(f"b{b}_ln2"):
                o2T = _layernorm(c, g2, o1T, "ln2", F16)
            with nc.named_scope(f"b{b}_ffn"):
                f2 = _ffn(c, o2T)
            with nc.named_scope(f"b{b}_ln3"):
                o3T = _layernorm(c, f2, o2T, "ln3", F32)
            with nc.named_scope(f"b{b}_out"):
                _store_out3(c, o3T, out3[b])

    nc.compile()
    return nc


def _consts(c, io):
    nc = c.nc
    c.id16 = c.wp.tile([P, P], F16, tag="id16")
    make_identity(nc, c.id16[:])
    c.id32 = c.wp.tile([P, P], F32, tag="id32")
    make_identity(nc, c.id32[:])
    c.ones_row = c.wp.tile([1, P], F16, tag="ones_row")
    nc.vector.memset(c.ones_row[:], 1.0)
    c.ones_col = c.wp.tile([P, 1], F16, tag="ones_col")
    nc.vector.memset(c.ones_col[:], 1.0)
    c.epsb = c.wp.tile([1, 1], F32, tag="epsb")
    nc.vector.memset(c.epsb[:], EPS)

    # onehot4[i]: [128, 4] fp16, col i = 1 on every partition
    c.onehot4 = []
    for i in range(WAVE):
        t = c.wp.tile([P, WAVE], F16, tag=f"oh{i}")
        nc.gpsimd.memset(t[:], 1.0)
        nc.gpsimd.affine_select(
            out=t[:], in_=t[:], compare_op=OP.is_equal, fill=0.0,
            base=-i, pattern=[[1, WAVE]], channel_multiplier=0)
        c.onehot4.append(t)

    # pairsel4[j]: [4, 128] fp16: row 2j -> cols 0-63, row 2j+1 -> cols 64-127
    c.pairsel4 = []
    for j in range(WAVE // 2):
        t = c.wp.tile([WAVE, P], F16, tag=f"psel{j}")
        nc.gpsimd.memset(t[:], 1.0)
        nc.gpsimd.affine_select(
            out=t[:], in_=t[:], compare_op=OP.is_ge, fill=0.0,
            base=128 * j, pattern=[[1, P]], channel_multiplier=-64)
        nc.gpsimd.affine_select(
            out=t[:], in_=t[:], compare_op=OP.is_ge, fill=0.0,
            base=63 - 128 * j, pattern=[[-1, P]], channel_multiplier=64)
        c.pairsel4.append(t)

    # self-attn additive mask tiles (-1e9 * mask), both orientations, fp16
    c.mqk, c.mkqT = [], []
    if not c.mask_zero:
        for qc in range(DC):
            mf = c.f32s.tile([P, T], F32, tag="f32s")
            nc.sync.dma_start(mf[:], io["lam"][ts(qc, P), :])
            m16 = c.wp.tile([P, T], F16, tag=f"mqk{qc}")
            nc.vector.tensor_scalar_mul(m16[:], mf[:], -30000.0)
            c.mqk.append(m16)
        for kc in range(DC):
            mkq = c.wp.tile([P, T], F16, tag=f"mkqT{kc}", name=f"mkqT{kc}")
            c.mkqT.append(mkq)
        for qc in range(DC):
            for kc in range(DC):
                pt = c.psm.tile([P, P], F16, tag="psm")
                nc.tensor.transpose(pt[:], c.mqk[qc][:, ts(kc, P)], c.id16[:])
                nc.vector.tensor_copy(c.mkqT[kc][:, ts(qc, P)], pt[:])


def _weights(c, io):
    nc = c.nc
    c.w16, c.bcol, c.bvrow = {}, {}, {}
    for w in WNAMES:
        w16 = c.wp.tile([P, DC, D], F16, tag=w + "_16")
        for dc in range(DC):
            wf = c.ws.tile([P, D], F32, tag="ws512")
            nc.sync.dma_start(wf[:], io[w + "_w"][ts(dc, P), :])
            nc.vector.tensor_copy(w16[:, dc, :], wf[:])
        c.w16[w] = w16
        if w.endswith("wv"):
            c.bvrow[w] = _load_row16(c, io[w + "_b"], w + "_bvr")
        else:
            c.bcol[w] = _load_col(c, io[w + "_b"], DC, w + "_bc")
    c.ffw1 = c.wp.tile([P, DC, DFF], F16, tag="w1_16")
    for dc in range(DC):
        for fq in range(DFF // D):
            wf = c.ws.tile([P, D], F32, tag="ws512")
            nc.sync.dma_start(wf[:], io["ffn_w1"][ts(dc, P), ts(fq, D)])
            nc.vector.tensor_copy(c.ffw1[:, dc, ts(fq, D)], wf[:])
    c.ffw2 = c.wp.tile([P, FC, D], F16, tag="w2_16")
    for fc in range(FC):
        wf = c.ws.tile([P, D], F32, tag="ws512")
        nc.sync.dma_start(wf[:], io["ffn_w2"][ts(fc, P), :])
        nc.vector.tensor_copy(c.ffw2[:, fc, :], wf[:])
    c.b1col = _load_col(c, io["ffn_b1"], FC, "b1c")
    c.b2col = _load_col(c, io["ffn_b2"], DC, "b2c")
    c.lng = {ln: _load_col(c, io[ln + "_g"], DC, ln + "g")
             for ln in ("ln1", "ln2", "ln3")}
    c.lnb = {ln: _load_col(c, io[ln + "_b"], DC, ln + "b")
             for ln in ("ln1", "ln2", "ln3")}


def _load_T(c, xdram, role):
    """[512, 512] fp32 DRAM -> 4 transposed fp16 tiles [128(d), 512(t)]."""
    nc = c.nc
    scr = c.dram.tile([T, D], F16, tag="scr")
    for tc_ in range(DC):
        nf = c.f32s.tile([P, D], F32, tag="f32s")
        nc.sync.dma_start(nf[:], xdram[ts(tc_, P), :])
        n16 = c.s16.tile([P, D], F16, tag="s16")
        nc.vector.tensor_copy(n16[:], nf[:])
        nc.sync.dma_start(scr[ts(tc_, P), :], n16[:])
    out = []
    for dc in range(DC):
        tT = c.actT.tile([P, T], F16, tag=role)
        nc.sync.dma_start_transpose(tT[:], scr[:, ts(dc, P)])
        out.append(tT)
    return out


def _vproj(c, inT, wv16, bvrow):
    """V = in @ Wv + b, natural layout: 4 tiles [128(t), 512(f)] fp16."""
    nc = c.nc
    out = []
    for tc_ in range(DC):
        ps = c.pmm.tile([P, D], F32, tag="pmm")
        for dc in range(DC):
            nc.tensor.matmul(ps[:], inT[dc][:, ts(tc_, P)], wv16[:, dc, :],
                             start=(dc == 0), stop=False)
        nc.tensor.matmul(ps[:], c.ones_row[:], bvrow[:], start=False, stop=True)
        v = c.vp.tile([P, D], F16, tag="v")
        nc.vector.tensor_copy(v[:], ps[:])
        out.append(v)
    return out


def _proj_T(c, inT, w16, bcol, role, scale=None):
    """X-layout projection: out^T[f, t] = W.T @ in^T (+ b) [* scale]."""
    nc = c.nc
    out = []
    for fc in range(DC):
        ps = c.pmm.tile([P, T], F32, tag="pmm")
        for dc in range(DC):
            nc.tensor.matmul(ps[:], w16[:, dc, ts(fc, P)], inT[dc][:],
                             start=(dc == 0), stop=(dc == DC - 1))
        o = c.actT.tile([P, T], F16, tag=role)
        if scale is None:
            nc.vector.tensor_scalar_add(o[:], ps[:], bcol[:, fc:fc + 1])
        else:
            nc.vector.tensor_scalar(o[:], ps[:], bcol[:, fc:fc + 1], scale,
                                    op0=OP.add, op1=OP.mult)
        out.append(o)
    return out


def _attention(c, b, qinT, kinT, vnat, pfx, aw_out, self_mask, pad_nr, pad_ncol):
    """One MHA block. Returns G = gelu(O @ Wdo + b): 4 fp32 tiles, X layout."""
    nc = c.nc
    qT = _proj_T(c, qinT, c.w16[pfx + "_wq"], c.bcol[pfx + "_wq"], "qT",
                 scale=1.0 / np.sqrt(DH))
    kT = _proj_T(c, kinT, c.w16[pfx + "_wk"], c.bcol[pfx + "_wk"], "kT")
    v3 = [vnat[kc].rearrange("p (h d) -> p h d", h=H) for kc in range(DC)]
    self_mm_mask = self_mask and not c.mask_zero
    cross_mask = (not self_mask) and (pad_nr is not None)

    def hslice(tiles, h, col=None):
        t = tiles[h // 2]
        r0 = (h % 2) * DH
        if col is None:
            return t[r0:r0 + DH, :]
        return t[r0:r0 + DH, ts(col, P)]

    oT = []
    for w in range(H // WAVE):
        heads = [w * WAVE + i for i in range(WAVE)]
        # E^T[k, q] = exp(logits^T + mask) per head
        et = {}
        for kc in range(DC):
            for i, h in enumerate(heads):
                ps = c.pmm.tile([P, T], F32, tag="pmm")
                nc.tensor.matmul(ps[:], hslice(kT, h, kc), hslice(qT, h),
                                 start=True, stop=not self_mm_mask)
                if self_mm_mask:
                    nc.tensor.matmul(ps[:], c.id16[:], c.mkqT[kc][:],
                                     start=False, stop=True)
                e = c.e16.tile([P, T], F16, tag="e16")
                bias = pad_ncol[:, kc:kc + 1] if cross_mask else 0.0
                nc.scalar.activation(e[:], ps[:], AF.Exp, bias=bias)
                et[(i, kc)] = e

        # S[i, q] = sum_k E^T via one-hot matmuls into one [4, 512] psum
        psS = c.psm.tile([WAVE, T], F32, tag="psm")
        n = 0
        for i in range(WAVE):
            for kc in range(DC):
                nc.tensor.matmul(psS[:], c.onehot4[i][:], et[(i, kc)][:],
                                 start=(n == 0), stop=(n == WAVE * DC - 1))
                n += 1
        trows = c.smp.tile([WAVE, T], F32, tag="trows")
        nc.scalar.activation(trows[:], psS[:], AF.Ln)
        recip = c.smp.tile([WAVE, T], F16, tag="recip")
        nc.scalar.activation(recip[:], trows[:], AF.Exp, scale=-1.0)
        tcols = []
        for qc in range(DC):
            pt = c.psm.tile([P, WAVE], F32, tag="psm")
            nc.tensor.transpose(pt[:], trows[:, ts(qc, P)],
                                c.id32[0:WAVE, 0:WAVE])
            tcol = c.tcolp.tile([P, WAVE], F32, tag="tcol")
            nc.vector.tensor_scalar_mul(tcol[:], pt[:], -1.0)
            tcols.append(tcol)

        # O^T = (E^T / S) @ V per head pair (col-tiled concurrent matmuls)
        for j in range(WAVE // 2):
            h0, h1 = heads[2 * j], heads[2 * j + 1]
            psav = c.pmm.tile([P, T], F32, tag="pmm")
            for kc in range(DC):
                nc.tensor.matmul(psav[0:DH, :], v3[kc][:, h0, :],
                                 et[(2 * j, kc)][:],
                                 start=(kc == 0), stop=(kc == DC - 1),
                                 tile_position=(0, 0))
                nc.tensor.matmul(psav[DH:P, :], v3[kc][:, h1, :],
                                 et[(2 * j + 1, kc)][:],
                                 start=(kc == 0), stop=(kc == DC - 1),
                                 tile_position=(0, DH))
            psb = c.pbc.tile([P, T], F32, tag="pbc")
            nc.tensor.matmul(psb[:], c.pairsel4[j][:], recip[:],
                             start=True, stop=True)
            bsb = c.s16.tile([P, T], F16, tag="s16")
            nc.vector.tensor_copy(bsb[:], psb[:])
            o = c.otp.tile([P, T], F16, tag="oT")
            nc.vector.tensor_mul(o[:], psav[:], bsb[:])
            oT.append(o)

        # aw[q, k] = exp(L - ln S + mask) -> fp32 -> DMA
        for qc in range(DC):
            for i, h in enumerate(heads):
                ps = c.pmm.tile([P, T], F32, tag="pmm")
                need2 = self_mm_mask or cross_mask
                nc.tensor.matmul(ps[:], hslice(qT, h, qc), hslice(kT, h),
                                 start=True, stop=not need2)
                if self_mm_mask:
                    nc.tensor.matmul(ps[:], c.id16[:], c.mqk[qc][:],
                                     start=False, stop=True)
                elif cross_mask:
                    nc.tensor.matmul(ps[:], c.ones_row[:], pad_nr[:],
                                     start=False, stop=True)
                aw = c.awp.tile([P, T], F32, tag="aw")
                nc.scalar.activation(aw[:], ps[:], AF.Exp,
                                     bias=tcols[qc][:, i:i + 1])
                nc.sync.dma_start(aw_out[b, h, ts(qc, P), :], aw[:])

    # G^T = gelu(Wdo.T @ O^T + b)
    g = []
    wdo, bdo = c.w16[pfx + "_do"], c.bcol[pfx + "_do"]
    for fc in range(DC):
        ps = c.pmm.tile([P, T], F32, tag="pmm")
        for dc in range(DC):
            nc.tensor.matmul(ps[:], wdo[:, dc, ts(fc, P)], oT[dc][:],
                             start=(dc == 0), stop=(dc == DC - 1))
        gt = c.f32s.tile([P, T], F32, tag="f32s")
        nc.scalar.activation(gt[:], ps[:], AF.Gelu, bias=bdo[:, fc:fc + 1])
        g.append(gt)
    return g


def _layernorm(c, main, resid, ln, out_dt):
    """X-layout layernorm over D (partition axis) of (main + resid)."""
    nc = c.nc
    g, bb = c.lng[ln], c.lnb[ln]
    r32, r16 = [], []
    for dc in range(DC):
        r = c.f32s.tile([P, T], F32, tag="f32s")
        if resid is not None:
            nc.vector.tensor_add(r[:], main[dc][:], resid[dc][:])
        else:
            nc.vector.tensor_copy(r[:], main[dc][:])
        r32.append(r)
        hh = c.s16.tile([P, T], F16, tag="s16")
        nc.vector.tensor_copy(hh[:], r[:])
        r16.append(hh)
    pssum = c.psm.tile([1, T], F32, tag="psm")
    pssq = c.psm.tile([1, T], F32, tag="psm")
    for dc in range(DC):
        sq = c.s16.tile([P, T], F16, tag="s16")
        nc.vector.tensor_mul(sq[:], r16[dc][:], r16[dc][:])
        nc.tensor.matmul(pssum[:], c.ones_col[:], r16[dc][:],
                         start=(dc == 0), stop=(dc == DC - 1))
        nc.tensor.matmul(pssq[:], c.ones_col[:], sq[:],
                         start=(dc == 0), stop=(dc == DC - 1))
    mu = c.smp.tile([1, T], F32, tag="rowf")
    nc.vector.tensor_scalar_mul(mu[:], pssum[:], 1.0 / D)
    musq = c.smp.tile([1, T], F32, tag="rowf")
    nc.vector.tensor_mul(musq[:], mu[:], mu[:])
    var = c.smp.tile([1, T], F32, tag="rowf")
    nc.vector.scalar_tensor_tensor(var[:], pssq[:], 1.0 / D, musq[:],
                                   op0=OP.mult, op1=OP.subtract)
    lnv = c.smp.tile([1, T], F32, tag="rowf")
    nc.scalar.activation(lnv[:], var[:], AF.Ln, bias=c.epsb[:, 0:1])
    arow = c.smp.tile([1, T], F16, tag="row16")
    nc.scalar.activation(arow[:], lnv[:], AF.Exp, scale=-0.5)
    brow = c.smp.tile([1, T], F16, tag="row16")
    nc.vector.scalar_tensor_tensor(brow[:], mu[:], -1.0, arow[:],
                                   op0=OP.mult, op1=OP.mult)
    psa = c.pbc.tile([P, T], F32, tag="pbc")
    nc.tensor.matmul(psa[:], c.ones_row[:], arow[:], start=True, stop=True)
    asb = c.s16.tile([P, T], F16, tag="s16")
    nc.vector.tensor_copy(asb[:], psa[:])
    psb = c.pbc.tile([P, T], F32, tag="pbc")
    nc.tensor.matmul(psb[:], c.ones_row[:], brow[:], start=True, stop=True)
    bsb = c.s16.tile([P, T], F16, tag="s16")
    nc.vector.tensor_copy(bsb[:], psb[:])
    out = []
    for dc in range(DC):
        t1 = c.f32s.tile([P, T], F32, tag="f32s")
        nc.vector.tensor_mul(t1[:], r32[dc][:], asb[:])
        t2 = c.f32s.tile([P, T], F32, tag="f32s")
        nc.vector.tensor_add(t2[:], t1[:], bsb[:])
        if out_dt == F16:
            o = c.actT.tile([P, T], F16, tag=ln + "o", name=ln + "o")
        else:
            o = c.f32s.tile([P, T], F32, tag="f32s", name=ln + "o32")
        nc.scalar.activation(o[:], t2[:], AF.Identity,
                             bias=bb[:, dc:dc + 1], scale=g[:, dc:dc + 1])
        out.append(o)
    return out


def _ffn(c, o2T):
    """r = relu(o2 @ w1 + b1) @ w2 + b2, 4 fp32 tiles (no residual yet)."""
    nc = c.nc
    f1 = []
    for fc in range(FC):
        ps = c.pmm.tile([P, T], F32, tag="pmm")
        for dc in range(DC):
            nc.tensor.matmul(ps[:], c.ffw1[:, dc, ts(fc, P)], o2T[dc][:],
                             start=(dc == 0), stop=(dc == DC - 1))
        f = c.e16.tile([P, T], F16, tag="e16")
        nc.scalar.activation(f[:], ps[:], AF.Relu, bias=c.b1col[:, fc:fc + 1])
        f1.append(f)
    out = []
    for dc in range(DC):
        ps = c.pmm.tile([P, T], F32, tag="pmm")
        for fc in range(FC):
            nc.tensor.matmul(ps[:], c.ffw2[:, fc, ts(dc, P)], f1[fc][:],
                             start=(fc == 0), stop=(fc == FC - 1))
        r = c.f32s.tile([P, T], F32, tag="f32s")
        nc.vector.tensor_scalar_add(r[:], ps[:], c.b2col[:, dc:dc + 1])
        out.append(r)
    return out


def _store_out3(c, o3T, out_dram):
    nc = c.nc
    for tc_ in range(DC):
        nat = c.awp.tile([P, D], F32, tag="aw")
        for dc in range(DC):
            pt = c.psm.tile([P, P], F32, tag="psm")
            nc.tensor.transpose(pt[:], o3T[dc][:, ts(tc_, P)], c.id32[:])
            nc.vector.tensor_copy(nat[:, ts(dc, P)], pt[:])
        nc.sync.dma_start(out_dram[ts(tc_, P), :], nat[:])


_BUILD_CACHE = {}


def _get_nc(mask_zero, pad_zero):
    key = (mask_zero, pad_zero)
    if key not in _BUILD_CACHE:
        _BUILD_CACHE[key] = build(mask_zero, pad_zero)
    return _BUILD_CACHE[key]


def kernel(**inputs):
    x = np.ascontiguousarray(np.asarray(inputs["x"], dtype=np.float32))
    enc_fc = np.ascontiguousarray(np.asarray(inputs["enc_fc"], dtype=np.float32))
    enc_ff = np.ascontiguousarray(np.asarray(inputs["enc_ff"], dtype=np.float32))
    lam = np.ascontiguousarray(
        np.asarray(inputs["look_ahead_mask"], dtype=np.float32).reshape(T, T))
    pad = np.ascontiguousarray(
        np.asarray(inputs["padding_mask"], dtype=np.float32).reshape(B, T))

    mask_zero = not lam.any()
    pad_zero = not pad.any()
    nc = _get_nc(mask_zero, pad_zero)

    common = {"lam": lam}
    for w in WNAMES:
        common[w + "_w"] = np.ascontiguousarray(
            np.asarray(inputs[w + "_w"], dtype=np.float32))
        common[w + "_b"] = np.ascontiguousarray(
            np.asarray(inputs[w + "_b"], dtype=np.float32))
    for k in ("ffn_w1", "ffn_b1", "ffn_w2", "ffn_b2",
              "ln1_g", "ln1_b", "ln2_g", "ln2_b", "ln3_g", "ln3_b"):
        common[k] = np.ascontiguousarray(np.asarray(inputs[k], dtype=np.float32))

    in_maps = []
    for core in range(NCORES):
        s = slice(core * BPC, (core + 1) * BPC)
        m = dict(common)
        m["x"] = x[s]
        m["enc_fc"] = enc_fc[s]
        m["enc_ff"] = enc_ff[s]
        m["pad"] = np.ascontiguousarray(pad[s])
        in_maps.append(m)

    res = run_bass_kernel_spmd(nc, in_maps, core_ids=list(range(NCORES)))
    out3 = np.concatenate([r["out3"] for r in res.results], axis=0)
    aw1 = np.concatenate([r["aw1"] for r in res.results], axis=0)
    aw2 = np.concatenate([r["aw2"] for r in res.results], axis=0)
    kernel.last_exec_ns = res.exec_time_ns
    return out3, aw1, aw2
